# revision 1
# baseline (speedup 1.0000x reference)
import sys

if "/opt/trn_rl_repo" not in sys.path:
    sys.path.insert(0, "/opt/trn_rl_repo")

import os
import numpy as np
import ml_dtypes
NO_CC = os.environ.get("KNO_CC", "0") == "1"
NO_DS = os.environ.get("KNO_DS", "0") == "1"


BF16 = ml_dtypes.bfloat16

# Problem constants (nn_BiLSTM_77034533421798)
T_FULL = 512
B_FULL = 128
H = 400
G = 1600  # 4*H
BL = 32  # batch per core (4 quarters x 2 directions on 8 cores)
NEG = 1.0e9

# hidden-unit halves (uneven on purpose: makes PSUM halves land on
# disjoint bank pairs: half0 -> 4*256=1024 fp32 = banks 0-1, half1 -> 576 = banks 2-3)
HALVES = [(0, 256), (256, 400)]
PT = [0, 1, 3, 2]  # tau (i,f,o,g) -> pytorch gate-block (i,f,g,o)


def _perm_and_sigma():
    perm = np.empty(G, np.int64)
    sigma = np.zeros(G, bool)
    n = 0
    for (u0, u1) in HALVES:
        for tau in range(4):
            for u in range(u0, u1):
                perm[n] = PT[tau] * H + u
                sigma[n] = tau < 3
                n += 1
    return perm, sigma


PERM, SIGMA = _perm_and_sigma()
# psum column layout == stream column layout (no holes): half0 [0:1024), half1 [1024:1600)
SIG0 = (0, 768)       # sigma block half0 (i,f,o x 256)
G0 = (768, 1024)      # g block half0
SIG1 = (1024, 1456)   # sigma block half1 (i,f,o x 144)
G1 = (1456, 1600)     # g block half1
NBLOCKS = [(0, 512), (512, 512), (1024, 512), (1536, 64)]  # psum-bank-safe matmul col blocks
HCH = [(0, 128), (128, 256), (256, 384), (384, 400)]  # h contraction chunks


def _build_program(T):
    import concourse.bacc as bacc
    import concourse.mybir as mybir
    import concourse.bass as bass
    import concourse.tile as tile

    dt = mybir.dt
    TB = T * BL
    NROW = TB // 128  # row tiles for the xg GEMMs

    nc = bacc.Bacc("TRN2", target_bir_lowering=False, debug=False, num_devices=8)

    # ---------------- I/O ----------------
    XTA = nc.dram_tensor("XTA", [402, TB], dt.float32, kind="ExternalInput")
    W0S = nc.dram_tensor("W0S", [402, G], dt.float32, kind="ExternalInput")
    WH0 = nc.dram_tensor("WH0", [H, G], dt.bfloat16, kind="ExternalInput")
    W1S = nc.dram_tensor("W1S", [802, G], dt.bfloat16, kind="ExternalInput")
    WH1 = nc.dram_tensor("WH1", [H, G], dt.bfloat16, kind="ExternalInput")
    MO = nc.dram_tensor("MO", [2, TB], dt.bfloat16, kind="ExternalInput")
    I32 = nc.dram_tensor("I32", [BL, BL], dt.float32, kind="ExternalInput")
    SEL0 = nc.dram_tensor("SEL0", [128, 1], dt.float32, kind="ExternalInput")
    SEL1 = nc.dram_tensor("SEL1", [128, 1], dt.float32, kind="ExternalInput")
    OUT = nc.dram_tensor("OUT", [TB, H], dt.float32, kind="ExternalOutput")

    with tile.TileContext(nc) as tc:
        with (
            tc.tile_pool(name="dram", bufs=1, space="DRAM") as dp,
            tc.tile_pool(name="wres", bufs=1) as wres,     # resident weight streams
            tc.tile_pool(name="stat", bufs=3) as statp,    # GEMM stationaries
            tc.tile_pool(name="psum", bufs=2, space="PSUM") as psp,
            tc.tile_pool(name="work", bufs=2) as wk,
            tc.tile_pool(name="xgio", bufs=3) as xgio,
            tc.tile_pool(name="hts", bufs=2) as htp,
            tc.tile_pool(name="cst", bufs=1) as cst,
        ):
            Sigmoid = mybir.ActivationFunctionType.Sigmoid
            Tanh = mybir.ActivationFunctionType.Tanh

            # internal DRAM
            XG0 = dp.tile([TB, G], dt.float32, tag="XG0", name="XG0")
            XG1 = dp.tile([TB, G], dt.float32, tag="XG1", name="XG1")
            L0T = dp.tile([H, TB], dt.bfloat16, tag="L0T", name="L0T")
            NBLK = max(1, T // 64)
            EB_ = T // NBLK
            EXIN = dp.tile([NBLK, H, BL * EB_], dt.bfloat16, tag="EXIN", name="EXIN")
            EXO = dp.tile([NBLK, 2, H, BL * EB_], dt.bfloat16, tag="EXO", name="EXO")
            PEER = dp.tile([H, TB], dt.bfloat16, tag="PEER", name="PEER")

            # ---- residents ----
            i32 = cst.tile([BL, BL], dt.float32, tag="i32")
            nc.sync.dma_start(out=i32[:], in_=I32[:])
            sel0 = cst.tile([128, 1], dt.float32, tag="sel0")
            nc.sync.dma_start(out=sel0[:], in_=SEL0[:])
            sel1 = cst.tile([128, 1], dt.float32, tag="sel1")
            nc.sync.dma_start(out=sel1[:], in_=SEL1[:])

            wh0c = []
            for ci, (r0, r1) in enumerate(HCH):
                w = wres.tile([r1 - r0, G], dt.bfloat16, tag=f"whh{ci}")
                nc.sync.dma_start(out=w[:], in_=WH0[r0:r1, :])
                wh0c.append(w)

            w0c = []
            for ci, (r0, r1) in enumerate([(0, 128), (128, 256), (256, 384), (384, 402)]):
                w = wres.tile([r1 - r0, G], dt.float32, tag=f"w0c{ci}")
                nc.sync.dma_start(out=w[:], in_=W0S[r0:r1, :])
                w0c.append(w)

            # =========== Stage A: xg0 = x_aug @ W0 ===========
            def xg_gemm_tile(j, statc, wstream, XGDST):
                # statc: list of (sbuf_tile, nrows); wstream: matching stream tiles
                acc = psp.tile([128, G], dt.float32, tag="acc")
                first = True
                for (st, rows), w in zip(statc, wstream):
                    last = st is statc[-1][0]
                    for (c0, cw) in NBLOCKS:
                        nc.tensor.matmul(
                            acc[:, c0:c0 + cw], st[:rows, :], w[:rows, c0:c0 + cw],
                            start=first, stop=last,
                        )
                    first = False
                ev = xgio.tile([128, G], dt.float32, tag="xgev")
                nc.vector.tensor_copy(ev[:], acc[:])
                nc.sync.dma_start(out=XGDST[128 * j:128 * (j + 1), :], in_=ev[:])

            for j in range(NROW):
                cols = slice(128 * j, 128 * (j + 1))
                statc = []
                for ci, (r0, r1) in enumerate([(0, 128), (128, 256), (256, 384), (384, 402)]):
                    st = statp.tile([r1 - r0, 128], dt.float32, tag=f"st{ci}")
                    nc.sync.dma_start(out=st[:], in_=XTA[r0:r1, cols])
                    statc.append((st, r1 - r0))
                xg_gemm_tile(j, statc, w0c, XG0)

            # =========== recurrence ===========
            def recurrence(phase, XG, whc, write_l0, write_out):
                c = cst.tile([BL, 512], dt.float32, tag=f"c{phase}")
                nc.vector.memset(c[:], 0.0)
                hT = []
                for ci, (r0, r1) in enumerate(HCH):
                    t0 = htp.tile([r1 - r0, BL], dt.bfloat16, tag=f"hT{ci}")
                    nc.vector.memset(t0[:], 0.0)
                    hT.append(t0)

                for t in range(T):
                    xgt = xgio.tile([BL, G], dt.float32, tag="xgt")
                    nc.sync.dma_start(out=xgt[:], in_=XG[BL * t:BL * (t + 1), :])

                    acc = psp.tile([128, G], dt.float32, tag="acc")
                    # identity chunk first (start=True clears bank + deposits xg)
                    for (c0, cw) in NBLOCKS:
                        nc.tensor.matmul(acc[:BL, c0:c0 + cw], i32[:], xgt[:, c0:c0 + cw],
                                         start=True, stop=False)
                    for ci, (r0, r1) in enumerate(HCH):
                        rows = r1 - r0
                        for (c0, cw) in NBLOCKS:
                            nc.tensor.matmul(acc[:BL, c0:c0 + cw], hT[ci][:rows, :],
                                             whc[ci][:rows, c0:c0 + cw],
                                             start=False, stop=(ci == 3))

                    sig = wk.tile([BL, 1344], dt.float32, tag="sig")
                    gt = wk.tile([BL, H], dt.float32, tag="gt")
                    tct = wk.tile([BL, H], dt.float32, tag="tct")
                    t1t = wk.tile([BL, 256], dt.float32, tag="t1t")
                    t2t = wk.tile([BL, 256], dt.float32, tag="t2t")
                    hbf = wk.tile([BL, 512], dt.bfloat16, tag="hbf")
                    nc.gpsimd.memset(hbf[:, 400:512], 0.0)
                    hf = wk.tile([BL, 512], dt.float32, tag="hf")

                    newhT = []
                    for ci, (r0, r1) in enumerate(HCH):
                        newhT.append(htp.tile([r1 - r0, BL], dt.bfloat16, tag=f"hT{ci}", name=f"nhT{ci}"))

                    for half in range(2):
                        u0, u1 = HALVES[half]
                        W = u1 - u0
                        sb, gb = (SIG0, G0) if half == 0 else (SIG1, G1)
                        soff = 768 * half  # sig tile layout: [0:768) half0, [768:1344) half1
                        nc.scalar.activation(sig[:, soff:soff + 3 * W], acc[:BL, sb[0]:sb[1]], Sigmoid)
                        nc.scalar.activation(gt[:, u0:u1], acc[:BL, gb[0]:gb[1]], Tanh)
                        # t2 = sig_i * g~ ; t1 = sig_f * c ; c = t1 + t2
                        nc.vector.tensor_mul(t2t[:, :W], sig[:, soff:soff + W], gt[:, u0:u1])
                        nc.gpsimd.tensor_mul(t1t[:, :W], sig[:, soff + W:soff + 2 * W], c[:, u0:u1])
                        nc.vector.tensor_add(c[:, u0:u1], t1t[:, :W], t2t[:, :W])
                        nc.scalar.activation(tct[:, u0:u1], c[:, u0:u1], Tanh)
                        nc.vector.tensor_mul(hf[:, u0:u1], sig[:, soff + 2 * W:soff + 3 * W], tct[:, u0:u1])
                        nc.gpsimd.tensor_copy(hbf[:, u0:u1], hf[:, u0:u1])
                        # transpose the finished 128-col chunks of hbf
                        # (chunks 0,1 from half0; 2,3 from half1; xbar needs free dim % 128 == 0)
                        for ci in ((0, 1) if half == 0 else (2, 3)):
                            if ci == 3:
                                full = wk.tile([128, BL], dt.bfloat16, tag="htf3")
                                nc.sync.dma_start_transpose(full[:], hbf[:, 384:512])
                                nc.vector.tensor_copy(newhT[3][:], full[:16, :])
                            else:
                                nc.sync.dma_start_transpose(newhT[ci][:], hbf[:, 128 * ci:128 * ci + 128])

                    if write_l0:
                        for ci, (r0, r1) in enumerate(HCH):
                            rows = r1 - r0
                            nc.sync.dma_start(out=L0T[r0:r1, BL * t:BL * (t + 1)], in_=newhT[ci][:rows, :])
                            _rt = T - 1 - t
                            _jb, _jo = _rt // EB_, _rt % EB_
                            nc.sync.dma_start(out=EXIN[_jb, r0:r1, BL * _jo:BL * (_jo + 1)], in_=newhT[ci][:rows, :])
                    if write_out:
                        nc.sync.dma_start(out=OUT[BL * t:BL * (t + 1), :], in_=hf[:, :H])

                    hT = newhT

                    # pairwise exchange in blocks
                    EB = EB_
                    if write_l0 and (t + 1) % EB == 0:
                        k = (t + 1) // EB - 1
                        jc = (T // EB - 1) - k
                        cb = slice(BL * EB * jc, BL * EB * (jc + 1))
                        if not NO_CC:
                            nc.gpsimd.collective_compute(
                                "AllGather", mybir.AluOpType.bypass,
                                replica_groups=[[0, 1], [2, 3], [4, 5], [6, 7]],
                                ins=[EXIN[jc]], outs=[EXO[jc]],
                            )
                            # blend the two gathered halves into PEER (peer = (1-d)*rank0 + d*rank1... 
                            # for me: peer slice = EXO[1-d]; sel0 = 1 iff my peer is rank0)
                            mw = BL * EB
                            for mci, (mr0, mr1) in enumerate(HCH):
                                e0 = wk.tile([128, mw], dt.bfloat16, tag="exm0", name="e0")
                                e1 = wk.tile([128, mw], dt.bfloat16, tag="exm1", name="e1")
                                pm = wk.tile([128, mw], dt.bfloat16, tag="exmp", name="pm")
                                rows = mr1 - mr0
                                nc.sync.dma_start(out=e0[:rows, :], in_=EXO[jc, 0, mr0:mr1, :])
                                nc.sync.dma_start(out=e1[:rows, :], in_=EXO[jc, 1, mr0:mr1, :])
                                nc.vector.tensor_scalar_mul(pm[:rows, :], e1[:rows, :], sel1[:rows, :])
                                nc.vector.scalar_tensor_tensor(
                                    pm[:rows, :], e0[:rows, :], sel0[:rows, :], pm[:rows, :],
                                    mybir.AluOpType.mult, mybir.AluOpType.add)
                                nc.sync.dma_start(out=PEER[mr0:mr1, cb], in_=pm[:rows, :])

            recurrence(0, XG0, wh0c, write_l0=True, write_out=False)

            # =========== Stage C: xg1 ===========
            wh1c = []
            for ci, (r0, r1) in enumerate(HCH):
                w = wres.tile([r1 - r0, G], dt.bfloat16, tag=f"whh{ci}")
                nc.sync.dma_start(out=w[:], in_=WH1[r0:r1, :])
                wh1c.append(w)

            w1c = []
            w1rows = [(0, 128), (128, 256), (256, 384), (384, 400), (400, 528), (528, 656), (656, 784), (784, 800)]
            for ci, (r0, r1) in enumerate(w1rows):
                rows = r1 - r0 + (2 if ci == 3 else 0)
                w = wres.tile([rows, G], dt.bfloat16, tag=f"w1c{ci}")
                nc.sync.dma_start(out=w[:r1 - r0, :], in_=W1S[r0:r1, :])
                if ci == 3:
                    nc.sync.dma_start(out=w[16:18, :], in_=W1S[800:802, :])
                w1c.append(w)


            for j in range(NROW):
                cols = slice(128 * j, 128 * (j + 1))
                statc = []
                # own chunks (local time order)
                for ci, (r0, r1) in enumerate(HCH):
                    rows = r1 - r0 + (2 if ci == 3 else 0)
                    st = statp.tile([rows, 128], dt.bfloat16, tag=f"s1o{ci}")
                    nc.sync.dma_start(out=st[:r1 - r0, :], in_=L0T[r0:r1, cols])
                    if ci == 3:
                        nc.sync.dma_start(out=st[16:18, :], in_=MO[:, cols])
                    statc.append((st, rows))
                # peer chunks via dynamic slice on EXO dim0
                for ci, (r0, r1) in enumerate(HCH):
                    rows = r1 - r0
                    st = statp.tile([rows, 128], dt.bfloat16, tag=f"s1p{ci}")
                    if NO_DS:
                        nc.sync.dma_start(out=st[:], in_=L0T[r0:r1, cols])
                    else:
                        nc.sync.dma_start(out=st[:], in_=PEER[r0:r1, cols])
                    statc.append((st, rows))
                xg_gemm_tile(j, statc, w1c, XG1)

            recurrence(1, XG1, wh1c, write_l0=False, write_out=True)

    nc.compile()
    return nc


_PROG_CACHE = {}


def _get_program(T):
    if T not in _PROG_CACHE:
        _PROG_CACHE[T] = _build_program(T)
    return _PROG_CACHE[T]


def _prep_core_inputs(x, lengths, wdict, T):
    """Build per-core input maps. x: [T,B,400] f32, lengths: [B] int."""
    B = x.shape[1]
    mask = (np.arange(T)[:, None] < np.asarray(lengths)[None, :])  # [T,B]
    in_maps = []
    i32 = np.eye(BL, dtype=np.float32)
    for core in range(8):
        p, d = core // 2, core % 2
        bs = slice(BL * p, BL * (p + 1))
        xl = np.asarray(x[:, bs, :], np.float32)
        ml = mask[:, bs].astype(np.float32)
        if d:
            xl, ml = xl[::-1], ml[::-1]
        TB = T * BL
        xt = np.ascontiguousarray(xl.reshape(TB, 400).T)  # [400, TB]
        negrow = (NEG * (1.0 - ml)).reshape(1, TB)
        ones = np.ones((1, TB), np.float32)
        XTA = np.concatenate([xt, negrow, ones], 0).astype(np.float32)
        MOv = np.concatenate([negrow, ones], 0).astype(BF16)

        dd = "f" if d == 0 else "b"
        wi0, whh0 = wdict[f"w_ih_{dd}0"], wdict[f"w_hh_{dd}0"]
        bi0 = wdict[f"b_ih_{dd}0"] + wdict[f"b_hh_{dd}0"]
        wi1, whh1 = wdict[f"w_ih_{dd}1"], wdict[f"w_hh_{dd}1"]
        bi1 = wdict[f"b_ih_{dd}1"] + wdict[f"b_hh_{dd}1"]

        def stream0(wi, bi):
            # [402, 1600]: rows 0:400 = wi.T permuted cols; 400 = -1 on sigma; 401 = bias
            out = np.zeros((402, G), np.float32)
            out[:400, :] = wi[PERM, :].T
            out[400, :] = np.where(SIGMA, -1.0, 0.0)
            out[401, :] = bi[PERM]
            return out.astype(np.float32)

        def stream1(wi, bi):
            # rows 0:400 must contract with OWN-chain l0out features, rows
            # 400:800 with the PEER chain's. wi's columns are [f(400), b(400)].
            out = np.zeros((802, G), np.float32)
            own = wi[:, 400 * d:400 * d + 400]     # own-direction features
            peer = wi[:, 400 * (1 - d):400 * (1 - d) + 400]
            out[:400, :] = own[PERM, :].T
            out[400:800, :] = peer[PERM, :].T
            out[800, :] = np.where(SIGMA, -1.0, 0.0)
            out[801, :] = bi[PERM]
            return out.astype(BF16)

        in_maps.append({
            "XTA": XTA,
            "W0S": stream0(wi0, bi0),
            "WH0": np.ascontiguousarray(whh0[PERM, :].T).astype(BF16),
            "W1S": stream1(wi1, bi1),
            "WH1": np.ascontiguousarray(whh1[PERM, :].T).astype(BF16),
            "MO": MOv,
            "I32": i32,
            "SEL0": np.full((128, 1), float(d), np.float32),
            "SEL1": np.full((128, 1), 1.0 - float(d), np.float32),
        })
    return in_maps


def _run(x, lengths, wdict, T):
    from concourse.bass_utils import run_bass_kernel_spmd

    nc = _get_program(T)
    in_maps = _prep_core_inputs(x, lengths, wdict, T)
    res = run_bass_kernel_spmd(nc, in_maps, list(range(8)))
    B = x.shape[1]
    out = np.zeros((T, B, 2 * H), np.float32)
    for core in range(8):
        p, d = core // 2, core % 2
        hl = res.results[core]["OUT"].reshape(T, BL, H)
        if d:
            hl = hl[::-1]
        out[:, BL * p:BL * (p + 1), H * d:H * (d + 1)] = hl
    return out


def kernel(x, lengths, **weights):
    x = np.asarray(x, np.float32)
    lengths = np.asarray(lengths)
    wd = {k: np.asarray(v, np.float32) for k, v in weights.items()}
    return _run(x, lengths, wd, x.shape[0])

